# revision 1
# baseline (speedup 1.0000x reference)
"""TRN2 Bass kernel for nn_ComboFwdVecComp (B=4, S=512, C=V=128).

out[b,i,j,v] = tanh( sum_c ctx[b,i,c] * ( Wm[v,c]*ctx[b,j,c] + (W2-Wd)[v,c] )
                     + A[b,j,v] + btot[v] )
  A = ctx @ (W1+Wd).T  (j-dep affine part),  btot = b1+b2+bm+bd.
The i-dep affine part (ctx_i @ (W2-Wd).T) folds into the main GEMM via the
(W2-Wd).T rhs term; the j-dep part becomes the K=1 bias matmul row.

Output (4,512,512,128) f32 = 512 MiB -> memory-bound (HBM write dominated).

Sharding: 8 cores, core k handles b = k//2, i in [ (k%2)*256, +256 ).
Each core emits out_shard (256, 512, 128) = 64 MiB; host concatenates.

i-MAJOR orientation (vs the earlier j-major kernel): PSUM partition dim = i,
free dim = (j, v) j-major. For fixed i (partition), 16 consecutive j x 128 v
= 8 KiB is CONTIGUOUS in out[i,j,v], so each store-DMA descriptor moves 8 KiB
instead of 512 B (the j-major kernel's v-only runs), lifting the store off
the per-queue descriptor-rate ceiling (~85 GB/s/queue at 512 B runs).

Per-core structure: loop j-groups (jg = 32 consecutive j's = 8 quads), then
i-chunks (ic = 128 i's), then halves (4 banks = 16 j's):
  rhs'[c,(j,v)] = WmT[c,v]*ctx_j[c] + W2mdT[c,v] prepped on DVE two quads at
       a time ([C,1024] mult+add, broadcast APs), REUSED by both i-chunks.
  bias mm (K=1, N=512): ones^T @ browp_quad -> bank, strip-tiled on PE
       row-strips 0/32/64/96 so the four bias mms run concurrently.
  main mm (K=128, N=512): ctxiT_chunk^T @ rhs'_quad accumulates on top.
  ACT tanh drains the half [128,2048] -> SBUF; ONE 1 MiB DMA stores it,
  rotating across the 3 queues (SP-HWDGE / Pool-SWDGE / ACT-HWDGE).

Store-DMA shape is the whole game (output = 512 MiB, HBM-write floor
~187us at the ~358 GB/s per-core HBM cap):
  - HWDGE queues need a 3D AP with outer dim = 128 partitions to spread
    descriptors over the 16 SDMA engines; a 2D row-list AP executes the
    whole chain on ONE engine (~27 GB/s -- measured 3.5x slowdown).
    max_dma_last_dim=1024 gives [[S*V,128],[1024,2],[1,1024]]: 4 KiB
    descriptors, 8x fewer descriptor slots than the 512 B j-major layout.
  - SWDGE (gpsimd) shreds any AP itself: plain 2D form emits one 8 KiB
    descriptor per partition row, round-robined over the 16 engines.
  - bf16/fp16 operands (faster PE) were tried and NOT kept: the f32r PE
    cadence (~2.73us per 2048-col half) paces store issue at just the DMA
    drain rate; faster PE bursts stores, queues pile up, and descriptors
    stretch under HBM contention (bf16 217us, fp16 215us vs f32r 205us --
    and bf16 rel err 1.6e-2 nearly trips the 2e-2 gate; fp16 was 2e-3).

All matmuls run in float32r (TF32-like, ~1.5e-4 rel err). f32r operands are
DMA'd directly into f32r SBUF tiles (browp, ctxiT; DRAM tensors declared
f32r) -- bits are f32, PE rounds internally; rhs' gets f32r from its
producing DVE add, ones from a DVE copy.
browp rows live on partitions {0,32,64,96}: j-quad q -> partition (q%4)*32,
column block q//4 (K=1 matmul base rules + strip tiling).
Input DMA order per queue = modeled completion order (Tile bakes it into
semaphore waits): rhs'-prep deps (ctxT[:, :32], wmT, w2md) first, then
browp/ctxiT for the first mms, then the ctxT bulk. A dummy tanh at build
start preloads the ACT lookup table, which otherwise stalls the first
drain ~9us mid-pipeline.
"""

import sys
import types
from contextlib import ExitStack

import numpy as np

import concourse.bass as bass
import concourse.mybir as mybir
import concourse.tile as tile
from concourse import bacc
from concourse.bass_utils import run_bass_kernel_spmd

B, S, C, V = 4, 512, 128, 128
NCORES = 8
NI = 256          # i's per core
NQJ = S // 4      # j quads (128)
NJG = NQJ // 8    # j groups of 8 quads / 32 j's (16)

_F32 = mybir.dt.float32
_F32R = mybir.dt.float32r


def install_ntff_shim():
    """antenv.axon_hooks is absent on some images; shim it so trace=True works."""
    if "antenv.axon_hooks" in sys.modules:
        return
    try:
        from trn_agent_boot.trn_boot import _ntff_profile_via_ctypes
        hook = _ntff_profile_via_ctypes("/opt/axon/libaxon_pjrt.so")
    except Exception:
        hook = None
    mod = types.ModuleType("antenv.axon_hooks")
    mod.get_axon_ntff_profile_hook = lambda: hook
    mod.set_axon_ntff_profile_hook = lambda h: None
    sys.modules["antenv.axon_hooks"] = mod


def build_nc():
    nc = bacc.Bacc("TRN2", target_bir_lowering=False, debug=False)

    BPW = (NQJ // 4) * 512  # browp row width (16384)

    ctxT_d = nc.dram_tensor("ctxT", [C, S], _F32, kind="ExternalInput").ap()
    ctxiT_d = nc.dram_tensor("ctxiT", [C, NI], _F32R, kind="ExternalInput").ap()
    wmT_d = nc.dram_tensor("wmT", [C, V], _F32, kind="ExternalInput").ap()
    w2md_d = nc.dram_tensor("w2md", [C, V], _F32, kind="ExternalInput").ap()
    browp_d = nc.dram_tensor("browp", [4, BPW], _F32R, kind="ExternalInput").ap()
    out_d = nc.dram_tensor("out_shard", [NI, S, V], _F32, kind="ExternalOutput").ap()

    with tile.TileContext(nc) as tc, ExitStack() as ctx:
        singles = ctx.enter_context(tc.tile_pool(name="singles", bufs=1))
        rhs_pool = ctx.enter_context(tc.tile_pool(name="rhs", bufs=8))
        tmp_pool = ctx.enter_context(tc.tile_pool(name="tmp", bufs=3))
        psum_pool = ctx.enter_context(tc.tile_pool(name="psum", bufs=1, space="PSUM"))
        out_pool = ctx.enter_context(tc.tile_pool(name="outs", bufs=6))

        # ---- load constants. Queue order = modeled completion order (the
        # Tile scheduler bakes it into semaphore waits): the first rhs'-prep
        # needs ctxT cols 0:32 + wmT + w2md, the first mms need browp rows
        # and ctxiT, so those go first on their queues. ----
        ctxT_sb = singles.tile([C, S], _F32)
        browp_r = singles.tile([97, BPW], _F32R)
        wmT_sb = singles.tile([C, V], _F32)
        w2md_sb = singles.tile([C, V], _F32)
        ctxiT_r = singles.tile([C, NI], _F32R)
        # browp rows first on both HWDGE queues: the first bias mms gate on
        # them and the Tile scheduler bakes modeled completion order into
        # semaphore waits (queueing them later pushed the first mm +4us).
        for r in range(4):
            eng = nc.sync if r % 2 == 0 else nc.scalar
            eng.dma_start(out=browp_r[32 * r:32 * r + 1, :], in_=browp_d[r:r + 1, :])
        nc.scalar.dma_start(out=ctxT_sb[:, 0:32], in_=ctxT_d[:, 0:32])
        nc.scalar.dma_start(out=wmT_sb, in_=wmT_d)
        nc.sync.dma_start(out=ctxiT_r, in_=ctxiT_d)
        nc.sync.dma_start(out=w2md_sb, in_=w2md_d)
        nc.scalar.dma_start(out=ctxT_sb[:, 32:], in_=ctxT_d[:, 32:])

        ones_f = singles.tile([97, 128], _F32)
        nc.vector.memset(ones_f, 1.0)
        ones_r = singles.tile([97, 128], _F32R)
        nc.vector.tensor_copy(ones_r, ones_f)
        # Dummy activation: the ACT engine loads its tanh lookup table on
        # first use (~9us stall observed mid-pipeline); trigger the load now
        # so it overlaps the input DMAs instead of stalling the first drain.
        warm = singles.tile([97, 8], _F32)
        nc.scalar.activation(
            warm, ones_f[:, 0:8], mybir.ActivationFunctionType.Tanh
        )

        # broadcast APs for pair-wide (8 j's) prep: wmT/w2md repeat over the
        # j dim (step 0), ctx_j scalars repeat over the v dim (trailing step 0)
        wm_b8 = bass.AP(
            tensor=wmT_sb.tensor,
            offset=wmT_sb.offset,
            ap=[wmT_sb.ap[0], [0, 8], wmT_sb.ap[1]],
        )
        w2md_b8 = bass.AP(
            tensor=w2md_sb.tensor,
            offset=w2md_sb.offset,
            ap=[w2md_sb.ap[0], [0, 8], w2md_sb.ap[1]],
        )

        # one 8-bank psum megatile; bank b occupies [:, b*512:(b+1)*512]
        P = psum_pool.tile([128, 4096], _F32, name="mega")

        dma_engines = [nc.sync, nc.gpsimd, nc.scalar]
        dma_i = 0

        def prep_pair(gp):
            # rhs' for j's [8*gp, 8*gp+8): one mult + one add over [C, 8*V]
            tmp_p = tmp_pool.tile([C, 8 * V], _F32)
            ctxj_bc = bass.AP(
                tensor=ctxT_sb.tensor,
                offset=ctxT_sb.offset + 8 * gp,
                ap=[ctxT_sb.ap[0], [1, 8], [0, V]],
            )
            nc.vector.tensor_tensor(
                out=tmp_p, in0=wm_b8, in1=ctxj_bc, op=mybir.AluOpType.mult
            )
            rhs_p = rhs_pool.tile([C, 8 * V], _F32R)
            nc.vector.tensor_tensor(
                out=rhs_p, in0=tmp_p, in1=w2md_b8, op=mybir.AluOpType.add
            )
            return rhs_p

        def pair_slice(pairs, qq):
            return pairs[qq // 2][:, (qq % 2) * 4 * V:(qq % 2 + 1) * 4 * V]

        for jg in range(NJG):
            if jg == 0:
                # ramp: only the first half's quads before the first matmuls
                pairs = [prep_pair(0), prep_pair(1), None, None]
            else:
                pairs = [prep_pair(4 * jg + pp) for pp in range(4)]

            for ic in range(2):
                for half in range(2):
                    # ---- bias mms: 4 quads, strip-concurrent ----
                    for s in range(4):
                        q = 8 * jg + 4 * half + s
                        strip = (q % 4) * 32
                        col = (q // 4) * 512
                        bank = 4 * half + s
                        nc.tensor.matmul(
                            P[:, bank * 512:(bank + 1) * 512],
                            lhsT=ones_r[strip:strip + 1, :],
                            rhs=browp_r[strip:strip + 1, col:col + 512],
                            start=True,
                            stop=False,
                            tile_position=(strip, 0),
                        )
                    # ---- main mms: one ctxiT LDW per half ----
                    for s in range(4):
                        bank = 4 * half + s
                        nc.tensor.matmul(
                            P[:, bank * 512:(bank + 1) * 512],
                            lhsT=ctxiT_r[:, ic * 128:(ic + 1) * 128],
                            rhs=pair_slice(pairs, 4 * half + s),
                            start=False,
                            stop=True,
                        )

                    if jg == 0 and ic == 0 and half == 0:
                        pairs[2] = prep_pair(2)
                        pairs[3] = prep_pair(3)

                    # ---- drain the half: tanh [128,2048] + ONE 1 MiB DMA ----
                    # HWDGE queues need a 3D AP (outer=128 partitions) to
                    # spread descriptors across the 16 SDMA engines -- a 2D
                    # row-list pins the whole chain on one engine.
                    # max_dma_last_dim=512 splits the 2048-elem run into
                    # [[512,4],[1,512]]: 2 KiB descriptors (vs the j-major
                    # kernel's 512 B), 4x fewer descriptor slots.
                    # SWDGE (gpsimd) shreds any shape into 512 B per-engine
                    # pieces itself, so it takes the plain 2D form.
                    ot = out_pool.tile([128, 2048], _F32)
                    nc.scalar.activation(
                        ot, P[:, half * 2048:(half + 1) * 2048],
                        mybir.ActivationFunctionType.Tanh,
                    )
                    j0 = jg * 32 + half * 16
                    dst = bass.AP(
                        tensor=out_d.tensor,
                        offset=(ic * 128) * S * V + j0 * V,
                        ap=[[S * V, 128], [1, 16 * V]],
                    )
                    eng = dma_engines[dma_i % 3]
                    dma_i += 1
                    if eng is nc.gpsimd:
                        eng.dma_start(out=dst, in_=ot[:, :])
                    else:
                        eng.dma_start(out=dst, in_=ot[:, :], max_dma_last_dim=1024)

    nc.compile()
    return nc


_NC_CACHE = {}


def get_nc():
    if "nc" not in _NC_CACHE:
        _NC_CACHE["nc"] = build_nc()
    return _NC_CACHE["nc"]


def make_in_maps(ctx, W1, b1, W2, b2, Wm, bm, Wd, bd):
    ctx = np.asarray(ctx, np.float32)
    btot = (
        np.asarray(b1) + np.asarray(b2) + np.asarray(bm) + np.asarray(bd)
    ).astype(np.float32)
    wmT = np.ascontiguousarray(np.asarray(Wm, np.float32).T)                  # (C,V)
    w2mdT = np.ascontiguousarray(
        (np.asarray(W2) - np.asarray(Wd)).T.astype(np.float32)
    )
    w1d = (np.asarray(W1) + np.asarray(Wd)).astype(np.float32)                # (V,C)

    per_b = []
    for b in range(B):
        A = (ctx[b] @ w1d.T + btot).astype(np.float32)                        # (S,V)
        browq = A.reshape(NQJ, 4 * V)                                         # quad rows
        browp = np.zeros((4, (NQJ // 4) * 512), np.float32)
        for q in range(NQJ):
            browp[q % 4, (q // 4) * 512:(q // 4) * 512 + 512] = browq[q]
        per_b.append((np.ascontiguousarray(ctx[b].T), browp))

    in_maps = []
    for k in range(NCORES):
        b = k // 2
        i0c = (k % 2) * NI
        ctxT, browp = per_b[b]
        in_maps.append({
            "ctxT": ctxT,
            "ctxiT": np.ascontiguousarray(ctx[b, i0c:i0c + NI].T),
            "wmT": wmT,
            "w2md": w2mdT,
            "browp": browp,
        })
    return in_maps


def run(in_maps, **kw):
    return run_bass_kernel_spmd(get_nc(), in_maps, core_ids=list(range(NCORES)), **kw)


def assemble(results):
    out = np.empty((B, S, S, V), np.float32)
    for k in range(NCORES):
        b = k // 2
        i0c = (k % 2) * NI
        out[b, i0c:i0c + NI] = results[k]["out_shard"]
    return out


def kernel(ctx, W1, b1, W2, b2, Wm, bm, Wd, bd):
    install_ntff_shim()
    in_maps = make_in_maps(ctx, W1, b1, W2, b2, Wm, bm, Wd, bd)
    res = run(in_maps)
    return assemble(res.results)

